# revision 17
# baseline (speedup 1.0000x reference)
"""Chamfer distance kernel for Trainium2 (8 NeuronCores, SPMD).

CD(P,Q) = mean_p min_q ||p-q||^2 + mean_q min_p ||q-p||^2
pc1: (4, 8192, 3) f32, pc2: (4, 8192, 3) f32 -> (4,) f32

Sharding: core c handles batch b = c//2, pc1-half h = c%2 (4096 rows).
Each core computes, for its (4096 x 8192) distance block:
  - rowmins: min over all pc2 for each of its 4096 pc1 rows  (d12 partials)
  - colmins: min over its 4096 pc1 rows for each pc2 point   (d21 partials)
Host combines: d12 = (sum of both halves' rowmins)/8192,
               d21 = mean(elementwise min of both halves' colmins).

Device algorithm (per core):
  dist(p,q) = ||p||^2 + ||q||^2 - 2 p.q is computed as a single K=18 bf16
  matmul via hi/lo bf16 splits of the coordinates and a 3-term bf16 split
  of the squared norms; every bf16 product is exact in the PE's fp32
  accumulator, so the distance matrix is fp32-accurate (~1e-6 abs).
  The 4 K-groups are replicated at SBUF partitions {0,32,64,96} so four
  N=512 matmuls run concurrently via tile_position row packing.
  Both orientations (pc1-major and pc2-major) of the distance matrix are
  produced; per 4-chunk iteration the DVE min-reduces one 512-wide PSUM
  tile directly and one 1536-wide tile staged to SBUF by the Scalar
  engine, via native tensor_scalar(op0=min(+big), op1=min, accum_out)
  ops (TENSOR_SCALAR_CACHE_REDUCE). Reduce-class DVE ops run 1x on this
  hardware, so the kernel is DVE-bound at ~1.25 ns/element; fused/custom
  InstISA ops (tensor_tensor_reduce etc.) fault on this runtime and
  GPSIMD tensor ops are rejected by walrus, which rules out the faster
  2-elem/cycle reduction structures.
"""

import functools
import sys

import numpy as np

sys.path.insert(0, "/opt/trn_rl_repo")

import ml_dtypes  # noqa: E402

import concourse.bacc as bacc  # noqa: E402
import concourse.tile as tile  # noqa: E402
from concourse import mybir  # noqa: E402
from concourse.bass_utils import run_bass_kernel_spmd  # noqa: E402

BF16 = ml_dtypes.bfloat16
N_CORES = 8
B, N1, N2 = 4, 8192, 8192
N1S = N1 // 2  # pc1 rows per core
K = 18  # augmented contraction dim


# --------------------------------------------------------------------------
# host-side prep: augmented bf16 arrays
# --------------------------------------------------------------------------
def _aug(pts: np.ndarray):
    """(N, 3) f32 -> (A, B): two (18, N) bf16 arrays such that
    sum_k A[k, p] * B[k, q] == || pts~[p] - pts~[q] ||^2 in exact arithmetic,
    where pts~ = hi+lo bf16 representation of pts (rel err ~2^-18)."""
    n = pts.shape[0]
    a_rows, b_rows = [], []
    eff = np.empty((n, 3), np.float64)
    for c in range(3):
        x = pts[:, c].astype(np.float32)
        hi = x.astype(BF16)
        lo = (x - hi.astype(np.float32)).astype(BF16)
        eff[:, c] = hi.astype(np.float64) + lo.astype(np.float64)
        m2h = (-2.0 * hi.astype(np.float32)).astype(BF16)  # exact (x2 of bf16)
        m2l = (-2.0 * lo.astype(np.float32)).astype(BF16)
        a_rows += [hi, hi, lo, lo]
        b_rows += [m2h, m2l, m2h, m2l]
    sq = (eff**2).sum(1)  # float64 squared norm of the *effective* points
    s1 = sq.astype(BF16)
    r = sq - s1.astype(np.float64)
    s2 = r.astype(BF16)
    s3 = (r - s2.astype(np.float64)).astype(BF16)
    one = np.ones(n, BF16)
    a_rows += [s1, s2, s3, one, one, one]
    b_rows += [one, one, one, s1, s2, s3]
    return np.ascontiguousarray(np.stack(a_rows)), np.ascontiguousarray(
        np.stack(b_rows)
    )


# --------------------------------------------------------------------------
# device program
# --------------------------------------------------------------------------
def _ts_min_reduce(nc, out_full, in_, acc):
    """DVE: out_full = in_ (via min with +big), acc = min-reduce(in_) per row."""
    nc.vector.tensor_scalar(
        out_full,
        in_,
        1.0e30,
        None,
        op0=mybir.AluOpType.min,
        op1=mybir.AluOpType.min,
        accum_out=acc,
    )


def _emit_phase(tc, pools, stat_sb, mov_sb, out_sb, n_blocks, n_iters):
    """One orientation: for each 128-row block of the stationary side,
    min-reduce over the full moving side (n_iters * 2048 points).

    Per iteration (4 N=512 matmuls, row-group packed) the 2048 distances
    land in a 4-bank PSUM tile which the Scalar engine stages to one half
    of a double-wide SBUF buffer; after two iterations the DVE runs a
    single 4096-wide tensor_scalar min-reduce over the buffer (wide ops
    amortize the ~140-cycle DVE op overhead and the 84 ns accumulator
    readout, and the 1x reduce is the hardware floor here). Per-pair
    partial mins collect in accbuf columns; one small reduce folds them
    into out_sb[:, r].
    """
    nc = tc.nc
    f32 = mybir.dt.float32
    n_cr = (n_iters + 1) // 2
    for r in range(n_blocks):
        if n_cr > 1:
            accbuf = pools["accbuf"].tile([128, n_cr], f32, tag="accbuf")
        st2 = None
        for j in range(n_iters):
            pb = pools["psumB"].tile([128, 2048], f32, tag="pb")
            for t in range(4):
                chunk = 4 * j + t
                nc.tensor.matmul(
                    pb[:, t * 512 : (t + 1) * 512],
                    lhsT=stat_sb[32 * t : 32 * t + K, r * 128 : (r + 1) * 128],
                    rhs=mov_sb[32 * t : 32 * t + K, chunk * 512 : (chunk + 1) * 512],
                    tile_position=(32 * t, 0),
                )
            if j % 2 == 0:
                st2 = pools["stage"].tile([128, 4096], f32, tag="stage")
            nc.scalar.copy(st2[:, (j % 2) * 2048 : (j % 2 + 1) * 2048], pb[:])
            if j % 2 == 1 or j == n_iters - 1:
                width = 4096 if j % 2 == 1 else 2048
                acc = (
                    out_sb[:, r : r + 1]
                    if n_cr == 1
                    else accbuf[:, j // 2 : j // 2 + 1]
                )
                outb = pools["outb"].tile([128, 4096], f32, tag="outb")
                _ts_min_reduce(nc, outb[:, :width], st2[:, :width], acc)
        if n_cr > 1:
            nc.vector.tensor_reduce(
                out_sb[:, r : r + 1],
                accbuf[:],
                axis=mybir.AxisListType.X,
                op=mybir.AluOpType.min,
            )


def _build(n1s=N1S, n2=N2):
    """Build + compile the (SPMD, identical on all cores) Bass program."""
    nc = bacc.Bacc("TRN2", target_bir_lowering=False, debug=False)

    bf = mybir.dt.bfloat16
    f32 = mybir.dt.float32
    a1 = nc.dram_tensor("a1", [K, n1s], bf, kind="ExternalInput").ap()
    b2 = nc.dram_tensor("b2", [K, n2], bf, kind="ExternalInput").ap()
    a2 = nc.dram_tensor("a2", [K, n2], bf, kind="ExternalInput").ap()
    b1 = nc.dram_tensor("b1", [K, n1s], bf, kind="ExternalInput").ap()
    blocks_12, pairs_12 = n1s // 128, n2 // 2048
    blocks_21, pairs_21 = n2 // 128, n1s // 2048
    rowmins = nc.dram_tensor("rowmins", [128, blocks_12], f32, kind="ExternalOutput")
    colmins = nc.dram_tensor("colmins", [128, blocks_21], f32, kind="ExternalOutput")

    with tile.TileContext(nc) as tc:
        with (
            tc.tile_pool(name="const", bufs=1) as const_pool,
            tc.tile_pool(name="outs", bufs=1) as out_pool,
            tc.tile_pool(name="stage", bufs=2) as stage_pool,
            tc.tile_pool(name="scr", bufs=1) as scr_pool,
            tc.tile_pool(name="small", bufs=3) as small_pool,
            tc.tile_pool(name="psumB", bufs=2, space="PSUM") as psum_b,
        ):
            a1s = const_pool.tile([128, n1s], bf, tag="a1s")
            b2s = const_pool.tile([128, n2], bf, tag="b2s")
            a2s = const_pool.tile([128, n2], bf, tag="a2s")
            b1s = const_pool.tile([128, n1s], bf, tag="b1s")
            for t in range(4):
                p0 = 32 * t
                nc.sync.dma_start(out=a1s[p0 : p0 + K, :], in_=a1)
                nc.sync.dma_start(out=b2s[p0 : p0 + K, :], in_=b2)
                nc.sync.dma_start(out=a2s[p0 : p0 + K, :], in_=a2)
                nc.sync.dma_start(out=b1s[p0 : p0 + K, :], in_=b1)

            rm_sb = out_pool.tile([128, blocks_12], f32, tag="rm")
            cm_sb = out_pool.tile([128, blocks_21], f32, tag="cm")

            pools = {
                "psumB": psum_b,
                "stage": stage_pool,
                "outb": scr_pool,
                "accbuf": small_pool,
            }
            _emit_phase(tc, pools, a1s, b2s, rm_sb, blocks_12, pairs_12)
            _emit_phase(tc, pools, a2s, b1s, cm_sb, blocks_21, pairs_21)

            nc.sync.dma_start(out=rowmins.ap(), in_=rm_sb[:])
            nc.sync.dma_start(out=colmins.ap(), in_=cm_sb[:])

    nc.compile()
    return nc


@functools.lru_cache(maxsize=2)
def _built(n1s, n2):
    return _build(n1s, n2)


# --------------------------------------------------------------------------
# entry point
# --------------------------------------------------------------------------
def _make_in_maps(pc1, pc2):
    in_maps = []
    for b in range(B):
        a1f, b1f = _aug(pc1[b])  # full pc1 of batch b, (18, 8192) each
        a2, b2 = _aug(pc2[b])
        for h in range(2):
            sl = slice(h * N1S, (h + 1) * N1S)
            in_maps.append(
                {
                    "a1": np.ascontiguousarray(a1f[:, sl]),
                    "b1": np.ascontiguousarray(b1f[:, sl]),
                    "a2": a2,
                    "b2": b2,
                }
            )
    return in_maps


def _combine(results):
    out = np.zeros(B, np.float32)
    for b in range(B):
        r0, r1 = results[2 * b], results[2 * b + 1]
        d12 = (
            r0["rowmins"].astype(np.float64).sum()
            + r1["rowmins"].astype(np.float64).sum()
        ) / N1
        d21 = np.minimum(r0["colmins"], r1["colmins"]).astype(np.float64).mean()
        out[b] = d12 + d21
    return out


def run(pc1, pc2, trace=False, **kw):
    nc = _built(N1S, N2)
    in_maps = _make_in_maps(np.asarray(pc1), np.asarray(pc2))
    res = run_bass_kernel_spmd(nc, in_maps, list(range(N_CORES)), trace=trace, **kw)
    return _combine(res.results), res


def kernel(pc1, pc2):
    out, _ = run(pc1, pc2)
    return out


# revision 18
# speedup vs baseline: 1.0094x; 1.0094x over previous
"""Chamfer distance kernel for Trainium2 (8 NeuronCores, SPMD).

CD(P,Q) = mean_p min_q ||p-q||^2 + mean_q min_p ||q-p||^2
pc1: (4, 8192, 3) f32, pc2: (4, 8192, 3) f32 -> (4,) f32

Sharding: core c handles batch b = c//2, pc1-half h = c%2 (4096 rows).
Each core computes, for its (4096 x 8192) distance block:
  - rowmins: min over all pc2 for each of its 4096 pc1 rows  (d12 partials)
  - colmins: min over its 4096 pc1 rows for each pc2 point   (d21 partials)
Host combines: d12 = (sum of both halves' rowmins)/8192,
               d21 = mean(elementwise min of both halves' colmins).

Device algorithm (per core):
  dist(p,q) = ||p||^2 + ||q||^2 - 2 p.q is computed as a single K=18 bf16
  matmul via hi/lo bf16 splits of the coordinates and a 3-term bf16 split
  of the squared norms; every bf16 product is exact in the PE's fp32
  accumulator, so the distance matrix is fp32-accurate (~1e-6 abs).
  The 4 K-groups are replicated at SBUF partitions {0,32,64,96} so four
  N=512 matmuls run concurrently via tile_position row packing.
  Both orientations (pc1-major and pc2-major) of the distance matrix are
  produced; per 4-chunk iteration the DVE min-reduces one 512-wide PSUM
  tile directly and one 1536-wide tile staged to SBUF by the Scalar
  engine, via native tensor_scalar(op0=min(+big), op1=min, accum_out)
  ops (TENSOR_SCALAR_CACHE_REDUCE). Reduce-class DVE ops run 1x on this
  hardware, so the kernel is DVE-bound at ~1.25 ns/element; fused/custom
  InstISA ops (tensor_tensor_reduce etc.) fault on this runtime and
  GPSIMD tensor ops are rejected by walrus, which rules out the faster
  2-elem/cycle reduction structures.
"""

import functools
import sys

import numpy as np

sys.path.insert(0, "/opt/trn_rl_repo")

import ml_dtypes  # noqa: E402

import concourse.bacc as bacc  # noqa: E402
import concourse.tile as tile  # noqa: E402
from concourse import mybir  # noqa: E402
from concourse.bass_utils import run_bass_kernel_spmd  # noqa: E402

BF16 = ml_dtypes.bfloat16
N_CORES = 8
B, N1, N2 = 4, 8192, 8192
N1S = N1 // 2  # pc1 rows per core
K = 18  # augmented contraction dim


# --------------------------------------------------------------------------
# host-side prep: augmented bf16 arrays
# --------------------------------------------------------------------------
def _aug(pts: np.ndarray):
    """(N, 3) f32 -> (A, B): two (18, N) bf16 arrays such that
    sum_k A[k, p] * B[k, q] == || pts~[p] - pts~[q] ||^2 in exact arithmetic,
    where pts~ = hi+lo bf16 representation of pts (rel err ~2^-18)."""
    n = pts.shape[0]
    a_rows, b_rows = [], []
    eff = np.empty((n, 3), np.float64)
    for c in range(3):
        x = pts[:, c].astype(np.float32)
        hi = x.astype(BF16)
        lo = (x - hi.astype(np.float32)).astype(BF16)
        eff[:, c] = hi.astype(np.float64) + lo.astype(np.float64)
        m2h = (-2.0 * hi.astype(np.float32)).astype(BF16)  # exact (x2 of bf16)
        m2l = (-2.0 * lo.astype(np.float32)).astype(BF16)
        a_rows += [hi, hi, lo, lo]
        b_rows += [m2h, m2l, m2h, m2l]
    sq = (eff**2).sum(1)  # float64 squared norm of the *effective* points
    s1 = sq.astype(BF16)
    r = sq - s1.astype(np.float64)
    s2 = r.astype(BF16)
    s3 = (r - s2.astype(np.float64)).astype(BF16)
    one = np.ones(n, BF16)
    a_rows += [s1, s2, s3, one, one, one]
    b_rows += [one, one, one, s1, s2, s3]
    return np.ascontiguousarray(np.stack(a_rows)), np.ascontiguousarray(
        np.stack(b_rows)
    )


# --------------------------------------------------------------------------
# device program
# --------------------------------------------------------------------------
def _ts_min_reduce(nc, out_full, in_, acc):
    """DVE: out_full = in_ (via min with +big), acc = min-reduce(in_) per row."""
    nc.vector.tensor_scalar(
        out_full,
        in_,
        1.0e30,
        None,
        op0=mybir.AluOpType.min,
        op1=mybir.AluOpType.min,
        accum_out=acc,
    )


def _emit_phase(tc, pools, stat_sb, mov_sb, out_sb, n_blocks, n_iters):
    """One orientation: for each 128-row block of the stationary side,
    min-reduce over the full moving side (n_iters * 2048 points).

    Per iteration (4 N=512 matmuls, row-group packed) the 2048 distances
    land in a 4-bank PSUM tile which the Scalar engine stages to one half
    of a double-wide SBUF buffer; after two iterations the DVE runs a
    single 4096-wide tensor_scalar min-reduce over the buffer (wide ops
    amortize the ~140-cycle DVE op overhead and the 84 ns accumulator
    readout, and the 1x reduce is the hardware floor here). Per-pair
    partial mins collect in accbuf columns; one small reduce folds them
    into out_sb[:, r].
    """
    nc = tc.nc
    f32 = mybir.dt.float32
    n_cr = (n_iters + 1) // 2
    for r in range(n_blocks):
        if n_cr > 1:
            accbuf = pools["accbuf"].tile([128, n_cr], f32, tag="accbuf")
        st2 = None
        for j in range(n_iters):
            pb = pools["psumB"].tile([128, 2048], f32, tag="pb")
            for t in range(4):
                chunk = 4 * j + t
                nc.tensor.matmul(
                    pb[:, t * 512 : (t + 1) * 512],
                    lhsT=stat_sb[32 * t : 32 * t + K, r * 128 : (r + 1) * 128],
                    rhs=mov_sb[32 * t : 32 * t + K, chunk * 512 : (chunk + 1) * 512],
                    tile_position=(32 * t, 0),
                )
            if j % 2 == 0:
                st2 = pools["stage"].tile([128, 4096], f32, tag="stage")
            nc.scalar.copy(st2[:, (j % 2) * 2048 : (j % 2 + 1) * 2048], pb[:])
            if j % 2 == 1 or j == n_iters - 1:
                width = 4096 if j % 2 == 1 else 2048
                acc = (
                    out_sb[:, r : r + 1]
                    if n_cr == 1
                    else accbuf[:, j // 2 : j // 2 + 1]
                )
                outb = pools["outb"].tile([128, 4096], f32, tag="outb")
                _ts_min_reduce(nc, outb[:, :width], st2[:, :width], acc)
        if n_cr > 1:
            nc.vector.tensor_reduce(
                out_sb[:, r : r + 1],
                accbuf[:],
                axis=mybir.AxisListType.X,
                op=mybir.AluOpType.min,
            )


def _build(n1s=N1S, n2=N2):
    """Build + compile the (SPMD, identical on all cores) Bass program."""
    nc = bacc.Bacc("TRN2", target_bir_lowering=False, debug=False)

    bf = mybir.dt.bfloat16
    f32 = mybir.dt.float32
    a1 = nc.dram_tensor("a1", [K, n1s], bf, kind="ExternalInput").ap()
    b2 = nc.dram_tensor("b2", [K, n2], bf, kind="ExternalInput").ap()
    a2 = nc.dram_tensor("a2", [K, n2], bf, kind="ExternalInput").ap()
    b1 = nc.dram_tensor("b1", [K, n1s], bf, kind="ExternalInput").ap()
    blocks_12, pairs_12 = n1s // 128, n2 // 2048
    blocks_21, pairs_21 = n2 // 128, n1s // 2048
    rowmins = nc.dram_tensor("rowmins", [128, blocks_12], f32, kind="ExternalOutput")
    colmins = nc.dram_tensor("colmins", [128, blocks_21], f32, kind="ExternalOutput")

    with tile.TileContext(nc) as tc:
        with (
            tc.tile_pool(name="const", bufs=1) as const_pool,
            tc.tile_pool(name="outs", bufs=1) as out_pool,
            tc.tile_pool(name="stage", bufs=2) as stage_pool,
            tc.tile_pool(name="scr", bufs=1) as scr_pool,
            tc.tile_pool(name="small", bufs=3) as small_pool,
            tc.tile_pool(name="psumB", bufs=2, space="PSUM") as psum_b,
        ):
            a1s = const_pool.tile([128, n1s], bf, tag="a1s")
            b2s = const_pool.tile([128, n2], bf, tag="b2s")
            a2s = const_pool.tile([128, n2], bf, tag="a2s")
            b1s = const_pool.tile([128, n1s], bf, tag="b1s")
            # d12's operands (a1, b2) first, split across the HWDGE (sync)
            # and SWDGE (gpsimd) queues so the first matmul pack's inputs
            # land in ~half the serial-DMA time; d21's operands follow.
            for t in range(4):
                p0 = 32 * t
                nc.sync.dma_start(out=a1s[p0 : p0 + K, :], in_=a1)
                nc.gpsimd.dma_start(out=b2s[p0 : p0 + K, :], in_=b2)
            for t in range(4):
                p0 = 32 * t
                nc.sync.dma_start(out=a2s[p0 : p0 + K, :], in_=a2)
                nc.gpsimd.dma_start(out=b1s[p0 : p0 + K, :], in_=b1)

            rm_sb = out_pool.tile([128, blocks_12], f32, tag="rm")
            cm_sb = out_pool.tile([128, blocks_21], f32, tag="cm")

            pools = {
                "psumB": psum_b,
                "stage": stage_pool,
                "outb": scr_pool,
                "accbuf": small_pool,
            }
            _emit_phase(tc, pools, a1s, b2s, rm_sb, blocks_12, pairs_12)
            _emit_phase(tc, pools, a2s, b1s, cm_sb, blocks_21, pairs_21)

            nc.sync.dma_start(out=rowmins.ap(), in_=rm_sb[:])
            nc.sync.dma_start(out=colmins.ap(), in_=cm_sb[:])

    nc.compile()
    return nc


@functools.lru_cache(maxsize=2)
def _built(n1s, n2):
    return _build(n1s, n2)


# --------------------------------------------------------------------------
# entry point
# --------------------------------------------------------------------------
def _make_in_maps(pc1, pc2):
    in_maps = []
    for b in range(B):
        a1f, b1f = _aug(pc1[b])  # full pc1 of batch b, (18, 8192) each
        a2, b2 = _aug(pc2[b])
        for h in range(2):
            sl = slice(h * N1S, (h + 1) * N1S)
            in_maps.append(
                {
                    "a1": np.ascontiguousarray(a1f[:, sl]),
                    "b1": np.ascontiguousarray(b1f[:, sl]),
                    "a2": a2,
                    "b2": b2,
                }
            )
    return in_maps


def _combine(results):
    out = np.zeros(B, np.float32)
    for b in range(B):
        r0, r1 = results[2 * b], results[2 * b + 1]
        d12 = (
            r0["rowmins"].astype(np.float64).sum()
            + r1["rowmins"].astype(np.float64).sum()
        ) / N1
        d21 = np.minimum(r0["colmins"], r1["colmins"]).astype(np.float64).mean()
        out[b] = d12 + d21
    return out


def run(pc1, pc2, trace=False, **kw):
    nc = _built(N1S, N2)
    in_maps = _make_in_maps(np.asarray(pc1), np.asarray(pc2))
    res = run_bass_kernel_spmd(nc, in_maps, list(range(N_CORES)), trace=trace, **kw)
    return _combine(res.results), res


def kernel(pc1, pc2):
    out, _ = run(pc1, pc2)
    return out
